# revision 16
# baseline (speedup 1.0000x reference)
"""DeepGO2 (MLP + GATConv + GO-embedding head) on 8 Trainium2 cores.

Wall-clock-optimized version. The device kernel runs in ~1 ms; the wall
time of kernel() is dominated by host work, the XLA/walrus compile, and
axon-relay transfers. Structure:

  * Static device program (edge-block counts fixed with ~6-sigma
    headroom over the binomial edge distribution), so the Bass build +
    NEFF compile is input-independent and runs in a background thread
    started at import time, overlapping reference-input staging.
  * Inputs shipped sharded: each core gets 1/8 of w1 / w2e / goT and
    the slices are re-assembled on device with AllGather collectives
    (saves ~200 MB of relay uplink vs replicating the tables).
  * Output quantized to uint8 on device (sigmoid output in [0,1]):
    105 MB pulled instead of 419 MB f32, dequantized on host.
  * No donated zero output buffers: the kernel writes every output
    byte, so the PJRT results are produced without shipping 105-419 MB
    of zeros up first.

Sharding: data-parallel over graph nodes. Each core owns 1250 nodes
(padded to 1280 = 10*128). Phase A computes the GAT projections for the
local node shard; an AllGather shares a per-node bf16/fp8 "payload"
table (h | el | q | 1); phase B does the edge-softmax aggregation for
the local dst shard with dma_gather + one-hot segment matmuls; phase C
is the [1280, 10240] logits matmul + sigmoid + uint8 quantization.

Math identities used (all host-precomputable):
  el = (x@fc_w)@attn_l = x@(fc_w@attn_l)        (and er, q likewise)
  logits[n,g] = sigmoid(agg_n[n]@go[g] + s[n] + rad'[g])
    s[n]    = agg_n[n]@hasFunc  (via payload column q = h@hasFunc)
    rad'[g] = |go_rad[g]| + gat_bias@go[g] + gat_bias@hasFunc
  edge softmax needs no max-subtraction: |e| <= ~2 for this data regime,
  exp() is computed unshifted and normalized by z = sum_e w_e.
"""

import os
import sys
import threading

for _p in ("/opt/trn_rl_repo", "/root/.axon_site/_ro/trn_rl_repo"):
    if os.path.isdir(_p) and _p not in sys.path:
        sys.path.insert(0, _p)

import time as _time

import numpy as np
import ml_dtypes

_T0 = _time.perf_counter()


def _log(msg):
    print(f"[kernel +{_time.perf_counter() - _T0:7.2f}s] {msg}", file=sys.stderr, flush=True)

# ---------------------------------------------------------------- constants
N, E, IN, H, G, NZ, R = 10000, 320000, 2560, 1024, 10000, 5000, 10
NC = 8            # cores
NPC = 1250        # real nodes per core
NT = 10           # node tiles per core
NPCP = NT * 128   # padded nodes per core (1280)
IN_T = IN // 128  # 20
H_T = H // 128    # 8
PAY = 1280        # payload row BYTES: h fp8 (1024B) | side bf16 (256B: el,q,one,pad)
W2C = H + 3       # fc_w | al2 | ar2 | q2
GP = 10240        # padded GO count
GPC = GP // NC    # GO columns per core shard (1280)
CB = 4            # blocks per dma_gather chunk (512 edges)
BF16 = ml_dtypes.bfloat16
QK = 253.0        # int8 quantization scale: q = QK*(sigmoid-0.5) + 0.5 in [-126, 127]
QOFF = 0.0        # dequant offset (sim-calibrated; convert rounds to nearest)

# Static per-tile edge-block counts (128 edges per block, groups of CB
# blocks per gather). Edges land in 80 (core, dst-tile) groups; counts
# are ~Binomial(320000, 1/80) -> mean 4000, sigma 63 for tiles 0..8 and
# mean 3136 for tile 9 (98 of 128 node columns real). 36/28 blocks give
# ~6.5 sigma of headroom; the data-dependent fallback rebuilds if ever
# exceeded.
NBLK_T = [36] * 9 + [28]
NBT = sum(NBLK_T)          # 352
EPC = NBT * 128            # 45056 padded edges per core

IN_NAMES = ["featT", "w1s", "w2es", "b1p", "goTs", "radp", "gidx", "dstl8"]


# ---------------------------------------------------------------- host prep
def _edge_prep(src, dst, nblk_t):
    """Edge scatter tables for the per-(core, dst-tile) gather layout.

    Returns (gidx_g [NC*128, EPC//16] int16, dstl_g [NC*128, NBT] int8)
    or None if a group exceeds the static capacity."""
    nbt_total = sum(nblk_t)
    epc = nbt_total * 128
    blk_base = np.zeros(NT + 1, np.int64)
    blk_base[1:] = np.cumsum(nblk_t)

    dstc = dst // NPC
    dloc = dst % NPC
    tl = dloc // 128
    dcol = dloc % 128
    group = dstc * NT + tl                 # [E] in [0, 80)
    counts = np.bincount(group, minlength=NC * NT).reshape(NC, NT)
    if (counts > np.asarray(nblk_t) * 128).any():
        return None

    order = np.argsort(group, kind="stable")
    g_s = group[order]
    src_s = src[order]
    dcol_s = dcol[order]
    gstart = np.zeros(NC * NT + 1, np.int64)
    gstart[1:] = np.cumsum(counts.reshape(-1))
    rank = np.arange(E, dtype=np.int64) - gstart[g_s]

    core_s = g_s // NT
    tile_s = g_s % NT
    slot = blk_base[tile_s] * 128 + rank   # slot within the core's padded edges
    srow = NPCP * (src_s // NPC) + (src_s % NPC)  # padded payload row of src

    gi = np.zeros((NC, epc), np.int16)
    gi[core_s, slot] = srow.astype(np.int16)
    dstloc = np.full((NC, nbt_total, 128), -1, np.int8)
    dstloc[core_s, slot // 128, slot % 128] = dcol_s.astype(np.int8)
    dstl_g = np.ascontiguousarray(dstloc.transpose(0, 2, 1)).reshape(NC * 128, nbt_total)

    # wrap gather indices: idx i -> [i % 16, i // 16], replicated to 128 rows
    gi_w = np.ascontiguousarray(
        np.tile(gi.reshape(NC, epc // 16, 16).transpose(0, 2, 1), (1, 8, 1))
    ).reshape(NC * 128, epc // 16)
    return gi_w, dstl_g


def _host_prep(inputs, nblk_t):
    """Build the global (concat-over-cores) input arrays, in IN_NAMES order.

    Returns None if the static edge capacity is exceeded (caller falls
    back to a dynamically sized build)."""
    f32 = np.float32
    features = np.asarray(inputs["features"], f32)
    src = np.asarray(inputs["src"]).astype(np.int64)
    dst = np.asarray(inputs["dst"]).astype(np.int64)
    W1 = np.asarray(inputs["W1"], f32)
    b1 = np.asarray(inputs["b1"], f32)
    fc_w = np.asarray(inputs["fc_w"], f32)
    attn_l = np.asarray(inputs["attn_l"], f32)
    attn_r = np.asarray(inputs["attn_r"], f32)
    gat_bias = np.asarray(inputs["gat_bias"], f32)
    go_embed = np.asarray(inputs["go_embed"], f32)
    go_rad = np.asarray(inputs["go_rad"], f32)
    rel_embed = np.asarray(inputs["rel_embed"], f32)

    edge = _edge_prep(src, dst, nblk_t)
    if edge is None:
        return None
    gidx_g, dstl_g = edge

    hf = rel_embed[R]                      # hasFunc row  [H]
    al2 = fc_w @ attn_l                    # [H]
    ar2 = fc_w @ attn_r
    q2 = fc_w @ hf
    w1_g = W1.astype(BF16)                 # global == concat of 8 row slices
    w2_g = np.concatenate(
        [fc_w, al2[:, None], ar2[:, None], q2[:, None]], axis=1
    ).astype(BF16)                         # [H, W2C]

    # goT ships WITHOUT hf: the hasFunc contribution arrives via the
    # payload q column (s[n] = agg@hf) on device
    goT = go_embed[:G].T.astype(BF16)      # [H, G]
    goT_g = np.zeros((NC * H, GPC), BF16)  # core c rows = goT cols [c*GPC:...]
    for c in range(NC):
        w = min(GPC, G - c * GPC)
        goT_g[c * H : (c + 1) * H, :w] = goT[:, c * GPC : c * GPC + w]

    rad_g = np.zeros((NC, GP), f32)
    rad_g[:, :G] = np.abs(go_rad[:G, 0]) + go_embed[:G] @ gat_bias + float(gat_bias @ hf)

    b1_g = np.tile(b1.reshape(H_T, 128).T, (NC, 1))  # [NC*128, H_T]

    fb = features.astype(BF16)
    ft_g = np.zeros((NC * IN, NPCP), BF16)
    for c in range(NC):
        ft_g[c * IN : (c + 1) * IN, :NPC] = fb[c * NPC : (c + 1) * NPC].T

    return [ft_g, w1_g, w2_g, b1_g, goT_g, rad_g, gidx_g, dstl_g]


# ---------------------------------------------------------------- device code
def build_nc(nblk_t):
    import concourse.bacc as bacc
    import concourse.mybir as mybir
    import concourse.tile as tile
    from concourse import library_config
    from concourse.masks import make_identity
    from concourse.tile_autobufs import add_dep_helper

    dt = mybir.dt
    AF = mybir.ActivationFunctionType
    ALU = mybir.AluOpType

    nbt_total = sum(nblk_t)
    epc = nbt_total * 128
    blk_base = [0]
    for nb in nblk_t:
        blk_base.append(blk_base[-1] + nb)

    nc = bacc.Bacc("TRN2", target_bir_lowering=False, debug=False, num_devices=NC)

    featT = nc.dram_tensor("featT", [IN, NPCP], dt.bfloat16, kind="ExternalInput")
    w1s = nc.dram_tensor("w1s", [IN // NC, H], dt.bfloat16, kind="ExternalInput")
    w2es = nc.dram_tensor("w2es", [H // NC, W2C], dt.bfloat16, kind="ExternalInput")
    b1p = nc.dram_tensor("b1p", [128, H_T], dt.float32, kind="ExternalInput")
    goTs = nc.dram_tensor("goTs", [H, GPC], dt.bfloat16, kind="ExternalInput")
    radp = nc.dram_tensor("radp", [1, GP], dt.float32, kind="ExternalInput")
    gidx = nc.dram_tensor("gidx", [128, epc // 16], dt.int16, kind="ExternalInput")
    dstl8 = nc.dram_tensor("dstl8", [128, nbt_total], dt.int8, kind="ExternalInput")
    out = nc.dram_tensor("out", [NPCP, GP], dt.int8, kind="ExternalOutput")

    # collectives cannot read IO tensors: bounce the input slices through
    # internal DRAM first
    w1b = nc.dram_tensor("w1b", [IN // NC, H], dt.bfloat16)
    w2b = nc.dram_tensor("w2b", [H // NC, W2C], dt.bfloat16)
    goTb = nc.dram_tensor("goTb", [H, GPC], dt.bfloat16)
    w1f = nc.dram_tensor("w1f", [IN, H], dt.bfloat16, addr_space="Shared")
    w2f = nc.dram_tensor("w2f", [H, W2C], dt.bfloat16, addr_space="Shared")
    goTf = nc.dram_tensor("goTf", [NC * H, GPC], dt.bfloat16, addr_space="Shared")
    pay_local = nc.dram_tensor("pay_local", [NPCP, PAY], dt.uint8)
    pay_full = nc.dram_tensor(
        "pay_full", [NC * NPCP, PAY], dt.uint8, addr_space="Shared"
    )

    grp = [list(range(NC))]

    with tile.TileContext(nc) as tc:
        lib_inst = nc.gpsimd.load_library(library_config.mlp)

        # table AllGathers: inputs are ready at kernel start
        d_w1b = nc.sync.dma_start(w1b[:], w1s[:])
        d_w2b = nc.sync.dma_start(w2b[:], w2es[:])
        d_gob = nc.sync.dma_start(goTb[:], goTs[:])
        cc_w1 = nc.gpsimd.collective_compute(
            "AllGather", ALU.bypass, replica_groups=grp, ins=[w1b[:]], outs=[w1f[:]]
        )
        add_dep_helper(cc_w1.ins, d_w1b.ins, sync=True, reason="ag after bounce")
        cc_w2 = nc.gpsimd.collective_compute(
            "AllGather", ALU.bypass, replica_groups=grp, ins=[w2b[:]], outs=[w2f[:]]
        )
        add_dep_helper(cc_w2.ins, d_w2b.ins, sync=True, reason="ag after bounce")
        cc_go = nc.gpsimd.collective_compute(
            "AllGather", ALU.bypass, replica_groups=grp, ins=[goTb[:]], outs=[goTf[:]]
        )
        add_dep_helper(cc_go.ins, d_gob.ins, sync=True, reason="ag after bounce")

        with (
            tc.tile_pool(name="const", bufs=1) as cp,
            tc.tile_pool(name="paydma", bufs=3) as paypool,
        ):
            ident = cp.tile([128, 128], dt.bfloat16)
            make_identity(nc, ident[:])
            ones1 = cp.tile([1, 128], dt.float32)
            nc.vector.memset(ones1[:], 1.0)
            ones1_bf = cp.tile([1, 128], dt.bfloat16)
            nc.vector.memset(ones1_bf[:], 1.0)
            iota_i = cp.tile([128, 128], dt.int32)
            nc.gpsimd.iota(iota_i[:], pattern=[[1, 128]], base=0, channel_multiplier=0)
            iota_bf = cp.tile([128, 128], dt.bfloat16)
            nc.vector.tensor_copy(iota_bf[:], iota_i[:])
            b1_sb = cp.tile([128, H_T], dt.float32)
            nc.sync.dma_start(b1_sb[:], b1p[:])
            er_sb = cp.tile([128, NT], dt.float32)
            er_bf = cp.tile([128, NT], dt.bfloat16)
            s_sb = cp.tile([128, NT], dt.float32)
            xg_sb = cp.tile([128, NT * H], dt.bfloat16)

            pay_dmas = []

            # ---------------- phase A: xT = relu(W1.T-ish), h_ext ----------
            with tc.tile_pool(name="phA", bufs=1) as ap:
                w1_sb = ap.tile([128, IN_T, H], dt.bfloat16)
                d_w1 = nc.sync.dma_start(
                    w1_sb[:], w1f.ap().rearrange("(k p) j -> p k j", p=128)
                )
                add_dep_helper(d_w1.ins, cc_w1.ins, sync=True, reason="w1 after ag")
                ft_sb = ap.tile([128, IN_T, NPCP], dt.bfloat16)
                nc.sync.dma_start(
                    ft_sb[:], featT.ap().rearrange("(k p) n -> p k n", p=128)
                )
                w2_sb = ap.tile([128, H_T, W2C], dt.bfloat16)
                d_w2 = nc.sync.dma_start(
                    w2_sb[:], w2f.ap().rearrange("(k p) j -> p k j", p=128)
                )
                add_dep_helper(d_w2.ins, cc_w2.ins, sync=True, reason="w2 after ag")
                xT_sb = ap.tile([128, H_T * NPCP], dt.bfloat16)

                with tc.tile_pool(name="psX", bufs=6, space="PSUM") as psx:
                    for j in range(H_T):
                        for fo in range(0, NPCP, 512):
                            fl = min(512, NPCP - fo)
                            ps = psx.tile([128, fl], dt.float32, tag="psx")
                            for k in range(IN_T):
                                nc.tensor.matmul(
                                    ps[:],
                                    w1_sb[:, k, j * 128 : (j + 1) * 128],
                                    ft_sb[:, k, fo : fo + fl],
                                    start=(k == 0),
                                    stop=(k == IN_T - 1),
                                )
                            nc.scalar.activation(
                                xT_sb[:, j * NPCP + fo : j * NPCP + fo + fl],
                                ps[:],
                                AF.Relu,
                                bias=b1_sb[:, j : j + 1],
                            )

                with (
                    tc.tile_pool(name="psH", bufs=3, space="PSUM") as psh_p,
                    tc.tile_pool(name="psS", bufs=2, space="PSUM") as pss_p,
                ):
                  for n in range(NT):
                    psh = psh_p.tile([128, H], dt.float32)
                    pss = pss_p.tile([128, 3], dt.float32)
                    for fo in range(0, H, 512):
                        for k in range(H_T):
                            nc.tensor.matmul(
                                psh[:, fo : fo + 512],
                                xT_sb[:, k * NPCP + n * 128 : k * NPCP + (n + 1) * 128],
                                w2_sb[:, k, fo : fo + 512],
                                start=(k == 0),
                                stop=(k == H_T - 1),
                            )
                    for k in range(H_T):
                        nc.tensor.matmul(
                            pss[:],
                            xT_sb[:, k * NPCP + n * 128 : k * NPCP + (n + 1) * 128],
                            w2_sb[:, k, H : H + 3],
                            start=(k == 0),
                            stop=(k == H_T - 1),
                        )
                    pay = paypool.tile([128, PAY], dt.uint8)
                    nc.vector.tensor_copy(
                        pay[:, 0:H].bitcast(dt.float8e4), psh[:]
                    )
                    side = pay[:, H:PAY].bitcast(dt.bfloat16)
                    nc.vector.tensor_copy(side[:, 0:1], pss[:, 0:1])
                    nc.vector.tensor_copy(side[:, 1:2], pss[:, 2:3])
                    nc.vector.memset(side[:, 2:3], 1.0)
                    nc.vector.memset(side[:, 3:128], 0.0)
                    nc.vector.tensor_copy(er_sb[:, n : n + 1], pss[:, 1:2])
                    d = nc.sync.dma_start(
                        pay_local[n * 128 : (n + 1) * 128, :], pay[:]
                    )
                    pay_dmas.append(d)
                nc.vector.tensor_copy(er_bf[:], er_sb[:])

            # ---------------- AllGather payload ---------------------------
            cc = nc.gpsimd.collective_compute(
                "AllGather",
                ALU.bypass,
                replica_groups=grp,
                ins=[pay_local[:]],
                outs=[pay_full[:]],
            )
            for d in pay_dmas:
                add_dep_helper(cc.ins, d.ins, sync=True, reason="cc after payload")

            # ---------------- phase B: edge aggregation -------------------
            with (
                tc.tile_pool(name="phB", bufs=1) as bp,
                tc.tile_pool(name="erbc", bufs=2) as ebp,
                tc.tile_pool(name="gat", bufs=5) as gp,
                tc.tile_pool(name="lw", bufs=4) as lwp,
                tc.tile_pool(name="psAgg", bufs=1, space="PSUM") as psagg,
                tc.tile_pool(name="psEr", bufs=2, space="PSUM") as pser,
                tc.tile_pool(name="small", bufs=4) as smp,
            ):
                gidx_sb = bp.tile([128, epc // 16], dt.int16)
                nc.sync.dma_start(gidx_sb[:], gidx[:])
                dl8_sb = bp.tile([128, nbt_total], dt.int8)
                nc.sync.dma_start(dl8_sb[:], dstl8[:])
                dl_sb = bp.tile([128, nbt_total], dt.float32)
                nc.vector.tensor_copy(dl_sb[:], dl8_sb[:])

                for t in range(NT):
                    nbt = nblk_t[t]
                    # er_bc[e, d] = er[tile t][d]  — 2-matmul partition broadcast
                    erp1 = pser.tile([1, 128], dt.float32, tag="erp1")
                    nc.tensor.matmul(erp1[:], er_bf[:, t : t + 1], ident[:])
                    erow = smp.tile([1, 128], dt.bfloat16, tag="erow")
                    nc.vector.tensor_copy(erow[:], erp1[:])
                    erp2 = pser.tile([128, 128], dt.float32, tag="erp2")
                    nc.tensor.matmul(erp2[:], ones1_bf[:], erow[:])
                    er_bc = ebp.tile([128, 128], dt.bfloat16, tag="erbc")
                    nc.vector.tensor_copy(er_bc[:], erp2[:])

                    ps0 = psagg.tile([128, 512], dt.float32, tag="agg0")
                    ps1 = psagg.tile([128, 512], dt.float32, tag="agg1")
                    psz = psagg.tile([128, 3], dt.float32, tag="aggz")

                    for c in range(nbt // CB):
                        gt = gp.tile([128, CB, PAY], dt.uint8, tag="gat")
                        icol = (blk_base[t] + c * CB) * 8
                        gd = nc.gpsimd.dma_gather(
                            gt[:],
                            pay_full[:],
                            gidx_sb[:, icol : icol + CB * 8],
                            CB * 128,
                            CB * 128,
                            PAY,
                        )
                        add_dep_helper(gd.ins, lib_inst.ins, sync=False,
                                       reason="gather after lib")
                        add_dep_helper(gd.ins, cc.ins, sync=True,
                                       reason="gather after allgather")
                        for b in range(CB):
                            blk = c * CB + b
                            # es = er_bc + el_src   (el rides in payload col H)
                            elf = lwp.tile([128, 1], dt.float32, tag="elf")
                            nc.vector.tensor_copy(
                                elf[:],
                                gt[:, b, H : H + 2].bitcast(dt.bfloat16),
                            )
                            es = lwp.tile([128, 128], dt.bfloat16, tag="es")
                            nc.vector.tensor_scalar_add(es[:], er_bc[:], elf[:])
                            # lr = leaky_relu(es) = max(0.2*es, es)
                            lr = lwp.tile([128, 128], dt.bfloat16, tag="lr")
                            nc.vector.scalar_tensor_tensor(
                                lr[:], es[:], 0.2, es[:], op0=ALU.mult, op1=ALU.max
                            )
                            # w = exp(lr)
                            wt = lwp.tile([128, 128], dt.bfloat16, tag="wt")
                            nc.scalar.activation(wt[:], lr[:], AF.Exp)
                            # lw = (iota == dstloc) * w
                            lw = lwp.tile([128, 128], dt.bfloat16, tag="lw")
                            nc.vector.scalar_tensor_tensor(
                                lw[:],
                                iota_bf[:],
                                dl_sb[:, blk_base[t] + blk : blk_base[t] + blk + 1],
                                wt[:],
                                op0=ALU.is_equal,
                                op1=ALU.mult,
                            )
                            first = blk == 0
                            last = blk == nbt - 1
                            h8 = gt[:, b, 0:H].bitcast(dt.float8e4)
                            sd = gt[:, b, H : H + 6].bitcast(dt.bfloat16)
                            nc.tensor.matmul(
                                ps0[:], lw[:], h8[:, 0:512],
                                start=first, stop=last,
                            )
                            nc.tensor.matmul(
                                ps1[:], lw[:], h8[:, 512:1024],
                                start=first, stop=last,
                            )
                            nc.tensor.matmul(
                                psz[:], lw[:], sd[:],
                                start=first, stop=last,
                            )

                    zc = smp.tile([128, 1], dt.float32, tag="zc")
                    nc.vector.tensor_scalar_max(zc[:], psz[:, 2:3], 1e-30)
                    rz = smp.tile([128, 1], dt.float32, tag="rz")
                    nc.vector.reciprocal(rz[:], zc[:])
                    nc.vector.tensor_tensor(
                        s_sb[:, t : t + 1], psz[:, 1:2], rz[:], op=ALU.mult
                    )
                    nc.scalar.mul(xg_sb[:, t * H : t * H + 512], ps0[:], rz[:])
                    nc.scalar.mul(xg_sb[:, t * H + 512 : (t + 1) * H], ps1[:], rz[:])

            # ---------------- phase C: logits ----------------------------
            with (
                tc.tile_pool(name="phC", bufs=1) as cpc,
                tc.tile_pool(name="goTp", bufs=2) as gop,
                tc.tile_pool(name="outp", bufs=4) as outp,
            ):
                rad_sb = cpc.tile([1, GP], dt.float32)
                nc.sync.dma_start(rad_sb[:], radp[:])
                rad_bc = cpc.tile([128, GP], dt.bfloat16)
                xgT_sb = cpc.tile([128, H_T * NPCP], dt.bfloat16)
                with tc.tile_pool(name="psT", bufs=4, space="PSUM") as pst_p:
                    for t in range(NT):
                        for k in range(H_T):
                            pst = pst_p.tile([128, 128], dt.bfloat16, tag="pst")
                            nc.tensor.transpose(
                                pst[:],
                                xg_sb[:, t * H + k * 128 : t * H + (k + 1) * 128],
                                ident[:],
                            )
                            nc.vector.tensor_copy(
                                xgT_sb[
                                    :, k * NPCP + t * 128 : k * NPCP + (t + 1) * 128
                                ],
                                pst[:],
                            )
                with tc.tile_pool(name="psC", bufs=8, space="PSUM") as psc_p:
                  for g2 in range(GP // 512):
                      psr = psc_p.tile([128, 512], dt.float32, tag="psc")
                      nc.tensor.matmul(
                          psr[:], ones1[:], rad_sb[:, g2 * 512 : (g2 + 1) * 512]
                      )
                      nc.vector.tensor_copy(
                          rad_bc[:, g2 * 512 : (g2 + 1) * 512], psr[:]
                      )
                  # goT arrives as 8 AllGather chunks: chunk ci rows
                  # [ci*H:(ci+1)*H] of goTf == goT columns [ci*GPC:(ci+1)*GPC]
                  for ci in range(NC):
                    goT_sb = gop.tile([128, H_T, GPC], dt.bfloat16, tag="goT")
                    d_go = nc.sync.dma_start(
                        goT_sb[:],
                        goTf.ap()[ci * H : (ci + 1) * H, :].rearrange(
                            "(k p) g -> p k g", p=128
                        ),
                    )
                    add_dep_helper(d_go.ins, cc_go.ins, sync=True,
                                   reason="goT after ag")
                    for n in range(NT):
                        u8 = outp.tile([128, GPC], dt.int8, tag="u8")
                        for go0, gl in ((0, 512), (512, 512), (1024, 256)):
                            ps = psc_p.tile([128, gl], dt.float32, tag="psc")
                            for k in range(H_T):
                                nc.tensor.matmul(
                                    ps[:],
                                    xgT_sb[
                                        :, k * NPCP + n * 128 : k * NPCP + (n + 1) * 128
                                    ],
                                    goT_sb[:, k, go0 : go0 + gl],
                                    start=(k == 0),
                                    stop=(k == H_T - 1),
                                )
                            g0 = ci * GPC + go0
                            st = outp.tile([128, gl], dt.bfloat16, tag="st")
                            nc.vector.scalar_tensor_tensor(
                                st[:],
                                ps[:],
                                s_sb[:, n : n + 1],
                                rad_bc[:, g0 : g0 + gl],
                                op0=ALU.add,
                                op1=ALU.add,
                            )
                            ot = outp.tile([128, gl], dt.float32, tag="ot")
                            nc.scalar.activation(ot[:], st[:], AF.Sigmoid)
                            # int8 quant: q = QK*sigmoid - (QK/2 - 0.5)
                            nc.scalar.activation(
                                u8[:, go0 : go0 + gl], ot[:], AF.Copy,
                                bias=0.5 - QK / 2.0, scale=QK,
                            )
                        nc.sync.dma_start(
                            out[n * 128 : (n + 1) * 128, ci * GPC : (ci + 1) * GPC],
                            u8[:],
                        )

    nc.compile()
    return nc


# ---------------------------------------------------------------- jit runner
def _make_compiled(nc, devices):
    import jax
    from jax.sharding import Mesh, PartitionSpec, NamedSharding
    from jax.experimental.shard_map import shard_map
    from concourse import mybir
    from concourse.bass2jax import (
        install_neuronx_cc_hook,
        _bass_exec_p,
        partition_id_tensor,
    )

    install_neuronx_cc_hook()
    partition_name = nc.partition_id_tensor.name

    in_names, out_names, out_avals, in_avals = [], [], [], []
    for alloc in nc.m.functions[0].allocations:
        if not isinstance(alloc, mybir.MemoryLocationSet):
            continue
        name = alloc.memorylocations[0].name
        if alloc.kind == "ExternalInput":
            if name != partition_name:
                in_names.append(name)
                in_avals.append(
                    (tuple(alloc.tensor_shape), mybir.dt.np(alloc.dtype))
                )
        elif alloc.kind == "ExternalOutput":
            out_names.append(name)
            out_avals.append(
                jax.core.ShapedArray(tuple(alloc.tensor_shape), mybir.dt.np(alloc.dtype))
            )
    assert in_names == IN_NAMES, in_names
    # outputs ride as donated zero-initialized operands (the NEFF binds its
    # output buffers to them); the zeros are created on-device by zfun so
    # no bytes cross the relay for them
    call_names = tuple(in_names) + tuple(out_names) + (partition_name,)
    n_params = len(in_names)
    n_outs = len(out_names)

    def _body(*args):
        operands = list(args)
        operands.append(partition_id_tensor())
        outs = _bass_exec_p.bind(
            *operands,
            out_avals=tuple(out_avals),
            in_names=call_names,
            out_names=tuple(out_names),
            lowering_input_output_aliases=(),
            sim_require_finite=True,
            sim_require_nnan=True,
            nc=nc,
        )
        return tuple(outs)

    mesh = Mesh(np.asarray(devices[:NC]), ("core",))
    spec = PartitionSpec("core")
    sharding = NamedSharding(mesh, spec)
    jf = jax.jit(
        shard_map(
            _body,
            mesh=mesh,
            in_specs=(spec,) * (n_params + n_outs),
            out_specs=(spec,) * n_outs,
            check_rep=False,
        ),
        donate_argnums=tuple(range(n_params, n_params + n_outs)),
        keep_unused=True,
    )
    gargs = [
        jax.ShapeDtypeStruct((NC * sh[0], *sh[1:]), dtp, sharding=sharding)
        for sh, dtp in in_avals
    ]
    gouts = [
        jax.ShapeDtypeStruct((NC * a.shape[0], *a.shape[1:]), a.dtype,
                             sharding=sharding)
        for a in out_avals
    ]
    _log("mc: lowering")
    lowered = jf.lower(*(gargs + gouts))
    _log("mc: lower done")
    compiled = lowered.compile()
    _log("mc: compile done")

    import jax.numpy as jnp

    zspecs = [((NC * a.shape[0], *a.shape[1:]), a.dtype) for a in out_avals]

    def _zeros():
        return tuple(jnp.zeros(shp, dtp) for shp, dtp in zspecs)

    zfun = jax.jit(_zeros, out_shardings=(sharding,) * n_outs).lower().compile()
    _log("mc: zfun done")
    return compiled, sharding, zfun


# ---------------------------------------------------------------- background
_bg = {
    "devices_ready": threading.Event(),
    "ready": threading.Event(),
    "lock": threading.Lock(),
    "thread": None,
}


def _bg_build():
    try:
        _log("bg: start")
        import jax

        devices = jax.devices()
        _bg["devices"] = devices
        _bg["devices_ready"].set()
        _log("bg: devices ready")
        nc = build_nc(NBLK_T)
        _log("bg: build_nc done")
        _bg["compiled"] = _make_compiled(nc, devices)
        _log("bg: compile done")
    except BaseException as e:  # noqa: BLE001
        _bg["error"] = e
    finally:
        _bg["devices_ready"].set()
        _bg["ready"].set()


def _ensure_bg():
    with _bg["lock"]:
        if _bg["thread"] is None:
            t = threading.Thread(target=_bg_build, daemon=True)
            _bg["thread"] = t
            t.start()


try:
    _ensure_bg()
except Exception:
    pass


# ---------------------------------------------------------------- entry point
def _sharding(devices):
    import jax
    from jax.sharding import Mesh, NamedSharding, PartitionSpec

    mesh = Mesh(np.asarray(devices[:NC]), ("core",))
    return NamedSharding(mesh, PartitionSpec("core"))


def _put_serial(arrays, sharding):
    # one transfer at a time: concurrent sharded uplinks interleave badly
    # on the relay (observed 0.5 MB/s aggregate vs ~50 MB/s serial)
    import jax

    w = jax.device_put(np.zeros((NC, 8), np.float32), sharding)
    jax.block_until_ready(w)
    _log("run: warmup put done")
    dev = []
    for a in arrays:
        d = jax.device_put(a, sharding)
        jax.block_until_ready(d)
        dev.append(d)
    _log("run: device_put done")
    return dev


def _exec_pull(compiled_pack, dev):
    import jax

    compiled, sharding, zfun = compiled_pack
    last = None
    for attempt in range(3):
        try:
            zeros = zfun()
            jax.block_until_ready(zeros)
            _log("run: zeros done")
            (out_g,) = compiled(*dev, *zeros)
            out_g.block_until_ready()
            _log("run: execute done")
            r = np.asarray(out_g)
            _log("run: pull done")
            return r
        except Exception as e:  # transient mesh desync: retry
            last = e
            _log(f"run: exec attempt {attempt} failed: {type(e).__name__}")
            _time.sleep(10.0 * (attempt + 1))
    raise last


def _run(compiled_pack, arrays):
    dev = _put_serial(arrays, compiled_pack[1])
    return _exec_pull(compiled_pack, dev)


def _dequant(q_global):
    # q = convert(QK*sigmoid - (QK/2 - 0.5)); sigmoid ~ (q + QK/2 - 0.5 + QOFF)/QK
    qs = np.arange(256, dtype=np.float32)
    qs[128:] -= 256.0  # index by int8 bits viewed as uint8
    lut = ((qs + QK / 2.0 - 0.5 + QOFF) * (1.0 / QK)).astype(np.float32)
    u8 = q_global.view(np.uint8) if q_global.dtype == np.int8 else q_global
    full = np.empty((N, G), np.float32)
    for c in range(NC):
        full[c * NPC : (c + 1) * NPC] = lut[u8[c * NPCP : c * NPCP + NPC, :G]]
    return full


def kernel(**inputs):
    _log("kernel: entry")
    _ensure_bg()
    arrays = _host_prep(inputs, NBLK_T)
    _log("kernel: host_prep done")

    if arrays is not None:
        # upload overlaps the tail of the background compile
        _bg["devices_ready"].wait()
        _log("kernel: devices ready")
        dev = None
        if _bg.get("devices") is not None and _bg.get("error") is None:
            try:
                dev = _put_serial(arrays, _sharding(_bg["devices"]))
            except Exception:
                dev = None
        _bg["ready"].wait()
        _log("kernel: bg ready")
        err = _bg.get("error")
        if err is None:
            if dev is not None:
                u8 = _exec_pull(_bg["compiled"], dev)
            else:
                u8 = _run(_bg["compiled"], arrays)
            r = _dequant(u8)
            _log("kernel: dequant done")
            return r
        # background build failed: retry synchronously once
        import jax

        devices = jax.devices()
        nc = build_nc(NBLK_T)
        u8 = _run(_make_compiled(nc, devices), arrays)
        return _dequant(u8)

    # static edge capacity exceeded: dynamically sized slow path
    import jax

    src = np.asarray(inputs["src"]).astype(np.int64)
    dst = np.asarray(inputs["dst"]).astype(np.int64)
    dstc = dst // NPC
    tl = (dst % NPC) // 128
    counts = np.bincount(dstc * NT + tl, minlength=NC * NT).reshape(NC, NT)
    maxcnt = counts.max(axis=0)
    nblk_t = [max(CB, ((int(m) + 127) // 128 + CB - 1) // CB * CB) for m in maxcnt]
    arrays = _host_prep(inputs, nblk_t)
    assert arrays is not None
    _bg["devices_ready"].wait()
    devices = _bg.get("devices")
    if devices is None:
        devices = jax.devices()
    nc = build_nc(nblk_t)
    u8 = _run(_make_compiled(nc, devices), arrays)
    return _dequant(u8)


if __name__ == "__main__":
    # quick self-run with random data (no reference check)
    rng = np.random.default_rng(0)
    ins = {
        "features": rng.standard_normal((N, IN), np.float32),
        "src": rng.integers(0, N, E),
        "dst": rng.integers(0, N, E),
        "W1": rng.standard_normal((IN, H), np.float32) * 0.02,
        "b1": np.zeros(H, np.float32),
        "fc_w": rng.standard_normal((H, H), np.float32) * 0.02,
        "attn_l": rng.standard_normal(H, np.float32) * 0.02,
        "attn_r": rng.standard_normal(H, np.float32) * 0.02,
        "gat_bias": np.zeros(H, np.float32),
        "go_embed": rng.standard_normal((G + NZ, H), np.float32) * 0.02,
        "go_rad": rng.standard_normal((G + NZ, 1), np.float32) * 0.02,
        "rel_embed": rng.standard_normal((R + 1, H), np.float32) * 0.02,
    }
    out = kernel(**ins)
    print("out", out.shape, out.dtype, out[:2, :4])
